# revision 8
# baseline (speedup 1.0000x reference)
"""CCPL contrastive loss kernel for Trainium2 (8 NeuronCores, SPMD data-parallel over batch).

Contract: kernel(**inputs) takes the FULL unsharded inputs and returns the FULL
scalar loss (float32, shape ()).

Strategy
--------
Only the top-left 32x32 corner of each feature map is ever read (sid in [0,30),
neighborhood offsets in {0,1,2}).  The host performs *indexing only* (gather of
neighbor/center columns from the corner; no arithmetic) and uploads, per core:

  xq, xk : [8*128, 576] packed K-chunks of [neigh(512) | center(64)] columns
  wts    : [128, WTOT]  packed transposed MLP weights (w1T / w2T chunks)
  aux    : [128, 268]   b1/b2 columns, identity block, ones block

Core b processes batch b end-to-end on device:
  x = neigh - center                      (VectorE, stride-0 broadcast AP)
  h = relu(w1 @ x + b1)                   (TensorE + ScalarE)
  y = w2 @ h + b2                         (TensorE + VectorE)
  f = y / (||y||_2 + 1e-7)                (ones-matmul partition reduction,
                                           sqrt via exp(0.5*ln), VectorE recip)
  G = f_q^T f_k                           (TensorE, |G|<=1 so exp needs no max)
  CE = ln(sum_t exp(G/tau)) - G[s,s]/tau  (ScalarE exp+accum, diag via
                                           tensor_tensor_reduce with I128)
Per-core partial sum of CE returned as [1,1]; host sums 8 partials / (8*512).
"""

import numpy as np
from contextlib import ExitStack

import concourse.bass as bass
import concourse.bacc as bacc
import concourse.tile as tile
from concourse import mybir
from concourse.bass_utils import run_bass_kernel_spmd

F32 = mybir.dt.float32

TAU = 0.07
NCORES = 8
S = 512          # 8 * num_s samples per batch-layer
NS = 64          # num_s
CS = [64, 128, 256, 512]
COUT = [16, 32, 64, 128]
KC = [1, 1, 2, 4]                 # 128-row K chunks per layer
NCH = sum(KC)                     # 8 chunks total in the x blob
_DH = np.array([0, 0, 0, 1, 1, 2, 2, 2], dtype=np.int64)
_DW = np.array([0, 1, 2, 0, 2, 0, 1, 2], dtype=np.int64)

# chunk bookkeeping -----------------------------------------------------------
CHUNK = {}
_c = 0
for _l in range(4):
    for _kk in range(KC[_l]):
        CHUNK[(_l, _kk)] = _c
        _c += 1

# weight blob column offsets
W1C, W2C = {}, {}
_c = 0
for _l in range(4):
    for _kk in range(KC[_l]):
        W1C[(_l, _kk)] = _c
        _c += CS[_l]
for _l in range(4):
    for _kk in range(KC[_l]):
        W2C[(_l, _kk)] = _c
        _c += COUT[_l]
WTOT = _c

# aux blob layout (f32): cols 0..7 b1 chunks, 8..11 b2, 12..139 I128, 140..267 ones
B1C = {}
_c = 0
for _l in range(4):
    for _m in range(KC[_l]):
        B1C[(_l, _m)] = _c
        _c += 1
B2C = {l: 8 + l for l in range(4)}
ICOL = 12
OCOL = 140
WVCOL = 268          # row 0: [1.0]*16 | [-1/tau]*4  (final combine weights)
AUXW = 288


def _build_nc(dt_x=F32):
    nc = bacc.Bacc()
    xq = nc.dram_tensor("xq", [NCH * 128, 576], dt_x, kind="ExternalInput")
    xk = nc.dram_tensor("xk", [NCH * 128, 576], dt_x, kind="ExternalInput")
    wts = nc.dram_tensor("wts", [128, WTOT], dt_x, kind="ExternalInput")
    aux = nc.dram_tensor("aux", [128, AUXW], F32, kind="ExternalInput")
    out = nc.dram_tensor("out", [1, 1], F32, kind="ExternalOutput")

    with ExitStack() as ctx:
        tc = ctx.enter_context(tile.TileContext(nc))
        const = ctx.enter_context(tc.tile_pool(name="const", bufs=1))
        work = ctx.enter_context(tc.tile_pool(name="work", bufs=2))
        fpool = ctx.enter_context(tc.tile_pool(name="fpool", bufs=4))
        ppool = ctx.enter_context(tc.tile_pool(name="psum", bufs=2, space="PSUM"))

        wall = const.tile([128, WTOT], dt_x)
        nc.sync.dma_start(out=wall, in_=wts[:, :])
        aall = const.tile([128, AUXW], F32)
        nc.sync.dma_start(out=aall, in_=aux[:, :])
        xq_s = const.tile([128, NCH, 576], dt_x)
        nc.sync.dma_start(out=xq_s, in_=xq.rearrange("(n p) m -> p n m", p=128))
        xk_s = const.tile([128, NCH, 576], dt_x)
        nc.sync.dma_start(out=xk_s, in_=xk.rearrange("(n p) m -> p n m", p=128))

        ones_col = aall[:, OCOL:OCOL + 1]
        # Z (row sums of exp(G/tau)) per G row-tile, one column per tile
        ZD = const.tile([128, 16], F32)
        # catb: cols 0..15 = per-tile sums of ln(Z); cols 16..19 = per-layer
        # sums of l_pos = sum(f_q * f_k)
        catb = const.tile([1, 20], F32)

        for l in range(4):
            C, Co, K = CS[l], COUT[l], KC[l]
            ftiles = []
            for xall in (xq_s, xk_s):
                # x = neigh - center (center broadcast over the 8 neighbors)
                xs = work.tile([128, K, S], dt_x, tag="xs")
                for kk in range(K):
                    cc = CHUNK[(l, kk)]
                    in0 = xall[:, cc, 0:512].rearrange("p (s j) -> p s j", j=8)
                    cb = xall[:, cc, 512:576]
                    in1 = bass.AP(cb.tensor, cb.offset, [*cb.ap, [0, 8]])
                    nc.vector.tensor_sub(
                        out=xs[:, kk, :].rearrange("p (s j) -> p s j", j=8),
                        in0=in0,
                        in1=in1,
                    )
                # h = relu(w1 @ x + b1), per 128-row output chunk
                h = work.tile([128, K, S], dt_x, tag="h")
                for m in range(K):
                    rows = min(128, C - m * 128)
                    mm1 = ppool.tile([128, S], F32, tag="mm1")
                    for kk in range(K):
                        c0 = W1C[(l, kk)] + m * 128
                        nc.tensor.matmul(
                            mm1[0:rows, :],
                            lhsT=wall[:, c0:c0 + rows],
                            rhs=xs[:, kk, :],
                            start=(kk == 0),
                            stop=(kk == K - 1),
                        )
                    bc1 = B1C[(l, m)]
                    nc.scalar.activation(
                        out=h[0:rows, m, :],
                        in_=mm1[0:rows, :],
                        func=mybir.ActivationFunctionType.Relu,
                        bias=aall[0:rows, bc1:bc1 + 1],
                        scale=1.0,
                    )
                # y = w2 @ h + b2
                mm2 = ppool.tile([128, S], F32, tag="mm2")
                for kk in range(K):
                    rows = min(128, C - kk * 128)
                    c0 = W2C[(l, kk)]
                    nc.tensor.matmul(
                        mm2[0:Co, :],
                        lhsT=wall[0:rows, c0:c0 + Co],
                        rhs=h[0:rows, kk, :],
                        start=(kk == 0),
                        stop=(kk == K - 1),
                    )
                y = work.tile([128, S], F32, tag="y")
                nc.vector.tensor_scalar_add(
                    out=y[0:Co, :], in0=mm2[0:Co, :],
                    scalar1=aall[0:Co, B2C[l]:B2C[l] + 1],
                )
                # ssq[s] = sum_c y^2  (partition reduction via ones matmul)
                y2 = work.tile([128, S], F32, tag="y2")
                nc.vector.tensor_mul(out=y2[0:Co, :], in0=y[0:Co, :], in1=y[0:Co, :])
                ssq = ppool.tile([1, S], F32, tag="small")
                nc.tensor.matmul(
                    ssq[:, :], lhsT=ones_col[0:Co, :], rhs=y2[0:Co, :],
                    start=True, stop=True,
                )
                # rn = 1 / (sqrt(ssq) + 1e-7); sqrt via exp(0.5*ln) to stay in
                # the exp/ln activation table set (ACT sqrt is low-precision).
                t1 = work.tile([1, S], F32, tag="t1")
                nc.scalar.activation(out=t1[:, :], in_=ssq[:, :],
                                     func=mybir.ActivationFunctionType.Ln)
                t2 = work.tile([1, S], F32, tag="t2")
                nc.scalar.activation(out=t2[:, :], in_=t1[:, :],
                                     func=mybir.ActivationFunctionType.Exp,
                                     scale=0.5)
                t3 = work.tile([1, S], F32, tag="t3")
                nc.vector.tensor_scalar_add(out=t3[:, :], in0=t2[:, :], scalar1=1e-7)
                rn = work.tile([1, S], F32, tag="rn")
                nc.vector.reciprocal(out=rn[:, :], in_=t3[:, :])
                # f = y * rn  (rn broadcast over partitions via K=1 ones matmul)
                bcp = ppool.tile([128, S], F32, tag="small")
                nc.tensor.matmul(
                    bcp[0:Co, :], lhsT=aall[0:1, OCOL:OCOL + Co], rhs=rn[:, :],
                    start=True, stop=True,
                )
                f = fpool.tile([128, S], F32, tag="f")
                nc.vector.tensor_mul(out=f[0:Co, :], in0=y[0:Co, :], in1=bcp[0:Co, :])
                ftiles.append(f)

            fq_t, fk_t = ftiles
            # sum of positive logits: sum_s <f_q[:,s], f_k[:,s]>
            pprod = work.tile([128, S], F32, tag="pprod")
            nc.vector.tensor_mul(out=pprod[0:Co, :], in0=fq_t[0:Co, :],
                                 in1=fk_t[0:Co, :])
            psum_pos = ppool.tile([1, S], F32, tag="small")
            nc.tensor.matmul(psum_pos[:, :], lhsT=ones_col[0:Co, :],
                             rhs=pprod[0:Co, :], start=True, stop=True)
            nc.vector.reduce_sum(out=catb[:, 16 + l:17 + l], in_=psum_pos[:, :],
                                 axis=mybir.AxisListType.X)
            for m in range(4):
                g = ppool.tile([128, S], F32, tag="g")
                nc.tensor.matmul(
                    g[:, :],
                    lhsT=fq_t[0:Co, m * 128:(m + 1) * 128],
                    rhs=fk_t[0:Co, :],
                    start=True, stop=True,
                )
                i = l * 4 + m
                # E = exp(G/tau); Z = row sum accumulated by ACT
                E = work.tile([128, S], F32, tag="E")
                nc.scalar.activation(
                    out=E[:, :], in_=g[:, :],
                    func=mybir.ActivationFunctionType.Exp,
                    scale=1.0 / TAU,
                    accum_out=ZD[:, i:i + 1],
                )

        # total = sum_{p,i} ln(Z) - (1/tau) * sum_l pos_l
        L = const.tile([128, 16], F32)
        nc.scalar.activation(out=L[:, :], in_=ZD[:, :],
                             func=mybir.ActivationFunctionType.Ln)
        tp = ppool.tile([1, 16], F32, tag="small")
        nc.tensor.matmul(tp[:, :], lhsT=ones_col, rhs=L[:, :], start=True, stop=True)
        nc.vector.tensor_copy(out=catb[:, 0:16], in_=tp[:, :])
        wprod = const.tile([1, 20], F32)
        nc.vector.tensor_mul(out=wprod[:, :], in0=catb[:, :],
                             in1=aall[0:1, WVCOL:WVCOL + 20])
        res = const.tile([1, 1], F32)
        nc.vector.reduce_sum(out=res[:, :], in_=wprod[:, :], axis=mybir.AxisListType.X)
        nc.sync.dma_start(out=out[:, :], in_=res[:, :])
    # bass2jax's PJRT path serializes nc.m directly without finalizing;
    # Bacc's legalization passes (matmul wait splitting, register
    # allocation) only run inside finalize().
    nc.finalize()
    return nc


_NC_CACHE = {}


def _get_nc(dt_x=F32):
    key = str(dt_x)
    if key not in _NC_CACHE:
        _NC_CACHE[key] = _build_nc(dt_x)
    return _NC_CACHE[key]


def _host_blobs(inputs, np_dt=np.float32):
    """Build the shared wts/aux blobs and the per-core xq/xk blobs."""
    # gather indices per layer (host-side indexing only)
    nidx, cidx = [], []
    for l in range(4):
        sid = np.asarray(inputs[f"sid{l}"]).astype(np.int64)
        nidx.append(((sid[:, 0:1] + _DH) * 32 + (sid[:, 1:2] + _DW)).reshape(-1))
        cidx.append((sid[:, 0] + 1) * 32 + (sid[:, 1] + 1))

    wts = np.zeros((128, WTOT), dtype=np_dt)
    aux = np.zeros((128, AUXW), dtype=np.float32)
    for l in range(4):
        w1T = np.asarray(inputs[f"w1_{l}"]).astype(np.float32).T  # [Cin, Cout]
        w2T = np.asarray(inputs[f"w2_{l}"]).astype(np.float32).T  # [Cin, Cout/4]
        b1 = np.asarray(inputs[f"b1_{l}"]).astype(np.float32)
        b2 = np.asarray(inputs[f"b2_{l}"]).astype(np.float32)
        C, Co = CS[l], COUT[l]
        for kk in range(KC[l]):
            rows = min(128, C - kk * 128)
            c0 = W1C[(l, kk)]
            wts[0:rows, c0:c0 + C] = w1T[kk * 128:kk * 128 + rows, :]
            c0 = W2C[(l, kk)]
            wts[0:rows, c0:c0 + Co] = w2T[kk * 128:kk * 128 + rows, :]
        for m in range(KC[l]):
            rows = min(128, C - m * 128)
            aux[0:rows, B1C[(l, m)]] = b1[m * 128:m * 128 + rows]
        aux[0:Co, B2C[l]] = b2
    aux[:, ICOL:ICOL + 128] = np.eye(128, dtype=np.float32)
    aux[:, OCOL:OCOL + 128] = 1.0
    aux[0, WVCOL:WVCOL + 16] = 1.0
    aux[0, WVCOL + 16:WVCOL + 20] = -1.0 / TAU

    # per-core x blobs: [NCH*128, 576] = packed [neigh | center] per K chunk
    xqs = [np.zeros((NCH * 128, 576), dtype=np_dt) for _ in range(NCORES)]
    xks = [np.zeros((NCH * 128, 576), dtype=np_dt) for _ in range(NCORES)]
    for l in range(4):
        C = CS[l]
        fq = np.asarray(inputs[f"fq{l}"])[:, :, :32, :32].reshape(NCORES, C, 1024)
        fk = np.asarray(inputs[f"fk{l}"])[:, :, :32, :32].reshape(NCORES, C, 1024)
        qn = fq[:, :, nidx[l]]   # [B, C, 512]
        qc = fq[:, :, cidx[l]]   # [B, C, 64]
        kn = fk[:, :, nidx[l]]
        kc_ = fk[:, :, cidx[l]]
        for b in range(NCORES):
            for kk in range(KC[l]):
                r0 = CHUNK[(l, kk)] * 128
                rows = min(128, C - kk * 128)
                sl = slice(kk * 128, kk * 128 + rows)
                xqs[b][r0:r0 + rows, 0:512] = qn[b, sl, :]
                xqs[b][r0:r0 + rows, 512:576] = qc[b, sl, :]
                xks[b][r0:r0 + rows, 0:512] = kn[b, sl, :]
                xks[b][r0:r0 + rows, 512:576] = kc_[b, sl, :]
    return wts, aux, xqs, xks


_LAST_RESULT = {}


def kernel(**inputs):
    assert int(inputs.get("start_layer", 0)) == 0
    assert int(inputs.get("end_layer", 4)) == 4
    assert int(inputs.get("num_s", 64)) == 64

    nc = _get_nc(F32)
    wts, aux, xqs, xks = _host_blobs(inputs, np.float32)
    in_maps = [
        {"xq": xqs[b], "xk": xks[b], "wts": wts, "aux": aux}
        for b in range(NCORES)
    ]
    r = run_bass_kernel_spmd(nc, in_maps, core_ids=list(range(NCORES)))
    _LAST_RESULT["r"] = r
    partials = [np.float64(r.results[b]["out"][0, 0]) for b in range(NCORES)]
    loss = np.float32(sum(partials) / (NCORES * S))
    return np.asarray(loss, dtype=np.float32)


# revision 16
# speedup vs baseline: 1.5799x; 1.5799x over previous
"""CCPL contrastive loss kernel for Trainium2 (8 NeuronCores, SPMD data-parallel over batch).

Contract: kernel(**inputs) takes the FULL unsharded inputs and returns the FULL
scalar loss (float32, shape ()).

Strategy
--------
Only the top-left 32x32 corner of each feature map is ever read (sid in [0,30),
neighborhood offsets in {0,1,2}).  The host performs *indexing only* (gather of
neighbor/center columns from the corner; no arithmetic) and uploads, per core:

  xq, xk : [8*128, 576] packed K-chunks of [neigh(512) | center(64)] columns
  wts    : [128, WTOT]  packed transposed MLP weights (w1T / w2T chunks)
  aux    : [128, 268]   b1/b2 columns, identity block, ones block

Core b processes batch b end-to-end on device:
  x = neigh - center                      (VectorE, stride-0 broadcast AP)
  h = relu(w1 @ x + b1)                   (TensorE + ScalarE)
  y = w2 @ h + b2                         (TensorE + VectorE)
  f = y / (||y||_2 + 1e-7)                (ones-matmul partition reduction,
                                           sqrt via exp(0.5*ln), VectorE recip)
  G = f_q^T f_k                           (TensorE, |G|<=1 so exp needs no max)
  CE = ln(sum_t exp(G/tau)) - G[s,s]/tau  (ScalarE exp+accum, diag via
                                           tensor_tensor_reduce with I128)
Per-core partial sum of CE returned as [1,1]; host sums 8 partials / (8*512).
"""

import numpy as np
from contextlib import ExitStack

import concourse.bass as bass
import concourse.bacc as bacc
import concourse.tile as tile
from concourse import mybir
from concourse.bass_utils import run_bass_kernel_spmd

F32 = mybir.dt.float32
F16 = mybir.dt.float16

# Force Exp/Ln/Relu to resolve to the one table set that contains all three
# (natural_log_exp_and_others), so the kernel pays a single ACT_TABLE_LOAD
# instead of thrashing between exp_and_others and natural_log (~1.3us each).
# Set ids stay aligned with act_info.json because only set CONTENTS are
# filtered, never the ordering.
_COMBINED_SET = "natural_log_exp_and_others"
_orig_get_tables = bacc.get_activation_tables


def _patched_get_tables(arch):
    t = _orig_get_tables(arch)
    strip = {
        mybir.ActivationFunctionType.Exp,
        mybir.ActivationFunctionType.Ln,
        mybir.ActivationFunctionType.Relu,
    }
    return {
        name: (fns if name == _COMBINED_SET else (set(fns) - strip))
        for name, fns in t.items()
    }


bacc.get_activation_tables = _patched_get_tables

TAU = 0.07
NCORES = 8
S = 512          # 8 * num_s samples per batch-layer
NS = 64          # num_s
CS = [64, 128, 256, 512]
COUT = [16, 32, 64, 128]
KC = [1, 1, 2, 4]                 # 128-row K chunks per layer
NCH = sum(KC)                     # 8 chunks total in the x blob
_DH = np.array([0, 0, 0, 1, 1, 2, 2, 2], dtype=np.int64)
_DW = np.array([0, 1, 2, 0, 2, 0, 1, 2], dtype=np.int64)

# chunk bookkeeping -----------------------------------------------------------
CHUNK = {}
_c = 0
for _l in range(4):
    for _kk in range(KC[_l]):
        CHUNK[(_l, _kk)] = _c
        _c += 1

# weight blob column offsets
W1C, W2C = {}, {}
_c = 0
for _l in range(4):
    for _kk in range(KC[_l]):
        W1C[(_l, _kk)] = _c
        _c += CS[_l]
for _l in range(4):
    for _kk in range(KC[_l]):
        W2C[(_l, _kk)] = _c
        _c += COUT[_l]
WTOT = _c

# aux blob layout (f32): cols 0..7 b1 chunks, 8..11 b2, 12..139 I128, 140..267 ones
B1C = {}
_c = 0
for _l in range(4):
    for _m in range(KC[_l]):
        B1C[(_l, _m)] = _c
        _c += 1
B2C = {l: 8 + l for l in range(4)}
ICOL = 12
OCOL = 140
WVCOL = 268          # row 0: [1.0]*16 | [-1/tau]*4  (final combine weights)
AUXW = 288


def _build_nc(dt_x=F16):
    nc = bacc.Bacc()
    xq = nc.dram_tensor("xq", [NCH * 128, 576], dt_x, kind="ExternalInput")
    xk = nc.dram_tensor("xk", [NCH * 128, 576], dt_x, kind="ExternalInput")
    wts = nc.dram_tensor("wts", [128, WTOT], dt_x, kind="ExternalInput")
    aux = nc.dram_tensor("aux", [128, AUXW], F32, kind="ExternalInput")
    auxh = nc.dram_tensor("auxh", [128, 128], F16, kind="ExternalInput")
    out = nc.dram_tensor("out", [1, 1], F32, kind="ExternalOutput")

    with ExitStack() as ctx:
        tc = ctx.enter_context(tile.TileContext(nc))
        const = ctx.enter_context(tc.tile_pool(name="const", bufs=1))
        work = ctx.enter_context(tc.tile_pool(name="work", bufs=2))
        fpool = ctx.enter_context(tc.tile_pool(name="fpool", bufs=4))
        ppool = ctx.enter_context(tc.tile_pool(name="psum", bufs=2, space="PSUM"))

        wall = const.tile([128, WTOT], dt_x)
        nc.sync.dma_start(out=wall, in_=wts[:, :])
        aall = const.tile([128, AUXW], F32)
        nc.sync.dma_start(out=aall, in_=aux[:, :])
        xq_s = const.tile([128, NCH, 576], dt_x)
        nc.sync.dma_start(out=xq_s, in_=xq.rearrange("(n p) m -> p n m", p=128))
        xk_s = const.tile([128, NCH, 576], dt_x)
        nc.sync.dma_start(out=xk_s, in_=xk.rearrange("(n p) m -> p n m", p=128))

        hall = const.tile([128, 128], F16)
        nc.sync.dma_start(out=hall, in_=auxh[:, :])

        ones_col = aall[:, OCOL:OCOL + 1]
        # Z (row sums of exp(G/tau)) per G row-tile, one column per tile
        ZD = const.tile([128, 16], F32)
        # catb: cols 0..15 = per-tile sums of ln(Z); cols 16..19 = per-layer
        # sums of l_pos = sum(f_q * f_k)
        catb = const.tile([1, 20], F32)

        for l in range(4):
            C, Co, K = CS[l], COUT[l], KC[l]
            ftiles = []
            for xall in (xq_s, xk_s):
                # x = neigh - center (center broadcast over the 8 neighbors)
                xs = work.tile([128, K, S], dt_x, tag="xs")
                for kk in range(K):
                    cc = CHUNK[(l, kk)]
                    in0 = xall[:, cc, 0:512].rearrange("p (s j) -> p s j", j=8)
                    cb = xall[:, cc, 512:576]
                    in1 = bass.AP(cb.tensor, cb.offset, [*cb.ap, [0, 8]])
                    nc.vector.tensor_sub(
                        out=xs[:, kk, :].rearrange("p (s j) -> p s j", j=8),
                        in0=in0,
                        in1=in1,
                    )
                # h = relu(w1 @ x + b1), per 128-row output chunk
                h = work.tile([128, K, S], dt_x, tag="h")
                for m in range(K):
                    rows = min(128, C - m * 128)
                    mm1 = ppool.tile([128, S], F32, tag="mm1")
                    for kk in range(K):
                        c0 = W1C[(l, kk)] + m * 128
                        nc.tensor.matmul(
                            mm1[0:rows, :],
                            lhsT=wall[:, c0:c0 + rows],
                            rhs=xs[:, kk, :],
                            start=(kk == 0),
                            stop=(kk == K - 1),
                        )
                    bc1 = B1C[(l, m)]
                    nc.scalar.activation(
                        out=h[0:rows, m, :],
                        in_=mm1[0:rows, :],
                        func=mybir.ActivationFunctionType.Relu,
                        bias=aall[0:rows, bc1:bc1 + 1],
                        scale=1.0,
                    )
                # y = w2 @ h + b2
                mm2 = ppool.tile([128, S], F32, tag="mm2")
                for kk in range(K):
                    rows = min(128, C - kk * 128)
                    c0 = W2C[(l, kk)]
                    nc.tensor.matmul(
                        mm2[0:Co, :],
                        lhsT=wall[0:rows, c0:c0 + Co],
                        rhs=h[0:rows, kk, :],
                        start=(kk == 0),
                        stop=(kk == K - 1),
                    )
                y = work.tile([128, S], F32, tag="y")
                nc.vector.tensor_scalar_add(
                    out=y[0:Co, :], in0=mm2[0:Co, :],
                    scalar1=aall[0:Co, B2C[l]:B2C[l] + 1],
                )
                # ssq[s] = sum_c y^2  (partition reduction via ones matmul)
                y2 = work.tile([128, S], F16, tag="y2")
                nc.vector.tensor_mul(out=y2[0:Co, :], in0=y[0:Co, :], in1=y[0:Co, :])
                ssq = ppool.tile([1, S], F32, tag="small")
                nc.tensor.matmul(
                    ssq[:, :], lhsT=hall[0:Co, 0:1], rhs=y2[0:Co, :],
                    start=True, stop=True,
                )
                # rn = 1/(sqrt(ssq) + 1e-7) entirely via exp/ln (single ACT
                # table set; DVE reciprocal costs ~3.3us per [1,512] op and
                # ACT sqrt is low-precision).
                t1 = work.tile([1, S], F32, tag="t1")
                nc.scalar.activation(out=t1[:, :], in_=ssq[:, :],
                                     func=mybir.ActivationFunctionType.Ln)
                t2 = work.tile([1, S], F32, tag="t2")
                nc.scalar.activation(out=t2[:, :], in_=t1[:, :],
                                     func=mybir.ActivationFunctionType.Exp,
                                     scale=0.5)
                t3 = work.tile([1, S], F32, tag="t3")
                nc.vector.tensor_scalar_add(out=t3[:, :], in0=t2[:, :], scalar1=1e-7)
                t4 = work.tile([1, S], F32, tag="t4")
                nc.scalar.activation(out=t4[:, :], in_=t3[:, :],
                                     func=mybir.ActivationFunctionType.Ln)
                rn = work.tile([1, S], F16, tag="rn")
                nc.scalar.activation(out=rn[:, :], in_=t4[:, :],
                                     func=mybir.ActivationFunctionType.Exp,
                                     scale=-1.0)
                # f = y * rn  (rn broadcast over partitions via K=1 ones matmul)
                bcp = ppool.tile([128, S], F32, tag="small")
                nc.tensor.matmul(
                    bcp[0:Co, :], lhsT=hall[0:1, 0:Co], rhs=rn[:, :],
                    start=True, stop=True,
                )
                f = fpool.tile([128, S], F16, tag="f")
                nc.vector.tensor_mul(out=f[0:Co, :], in0=y[0:Co, :], in1=bcp[0:Co, :])
                ftiles.append(f)

            fq_t, fk_t = ftiles
            # sum of positive logits: sum_s <f_q[:,s], f_k[:,s]>
            pprod = work.tile([128, S], F16, tag="pprod")
            nc.vector.tensor_mul(out=pprod[0:Co, :], in0=fq_t[0:Co, :],
                                 in1=fk_t[0:Co, :])
            psum_pos = ppool.tile([1, S], F32, tag="small")
            nc.tensor.matmul(psum_pos[:, :], lhsT=hall[0:Co, 0:1],
                             rhs=pprod[0:Co, :], start=True, stop=True)
            nc.vector.reduce_sum(out=catb[:, 16 + l:17 + l], in_=psum_pos[:, :],
                                 axis=mybir.AxisListType.X)
            for m in range(4):
                g = ppool.tile([128, S], F32, tag="g")
                nc.tensor.matmul(
                    g[:, :],
                    lhsT=fq_t[0:Co, m * 128:(m + 1) * 128],
                    rhs=fk_t[0:Co, :],
                    start=True, stop=True,
                )
                i = l * 4 + m
                # E = exp(G/tau); Z = row sum accumulated by ACT
                E = work.tile([128, S], F32, tag="E")
                nc.scalar.activation(
                    out=E[:, :], in_=g[:, :],
                    func=mybir.ActivationFunctionType.Exp,
                    scale=1.0 / TAU,
                    accum_out=ZD[:, i:i + 1],
                )

        # total = sum_{p,i} ln(Z) - (1/tau) * sum_l pos_l
        L = const.tile([128, 16], F32)
        nc.scalar.activation(out=L[:, :], in_=ZD[:, :],
                             func=mybir.ActivationFunctionType.Ln)
        tp = ppool.tile([1, 16], F32, tag="small")
        nc.tensor.matmul(tp[:, :], lhsT=ones_col, rhs=L[:, :], start=True, stop=True)
        nc.vector.tensor_copy(out=catb[:, 0:16], in_=tp[:, :])
        wprod = const.tile([1, 20], F32)
        nc.vector.tensor_mul(out=wprod[:, :], in0=catb[:, :],
                             in1=aall[0:1, WVCOL:WVCOL + 20])
        res = const.tile([1, 1], F32)
        nc.vector.reduce_sum(out=res[:, :], in_=wprod[:, :], axis=mybir.AxisListType.X)
        nc.sync.dma_start(out=out[:, :], in_=res[:, :])
    # bass2jax's PJRT path serializes nc.m directly without finalizing;
    # Bacc's legalization passes (matmul wait splitting, register
    # allocation) only run inside finalize().
    nc.finalize()
    return nc


_NC_CACHE = {}


def _get_nc(dt_x=F16):
    key = str(dt_x)
    if key not in _NC_CACHE:
        _NC_CACHE[key] = _build_nc(dt_x)
    return _NC_CACHE[key]


def _host_blobs(inputs, np_dt=np.float16):
    """Build the shared wts/aux blobs and the per-core xq/xk blobs."""
    # gather indices per layer (host-side indexing only)
    nidx, cidx = [], []
    for l in range(4):
        sid = np.asarray(inputs[f"sid{l}"]).astype(np.int64)
        nidx.append(((sid[:, 0:1] + _DH) * 32 + (sid[:, 1:2] + _DW)).reshape(-1))
        cidx.append((sid[:, 0] + 1) * 32 + (sid[:, 1] + 1))

    wts = np.zeros((128, WTOT), dtype=np_dt)
    aux = np.zeros((128, AUXW), dtype=np.float32)
    for l in range(4):
        w1T = np.asarray(inputs[f"w1_{l}"]).astype(np.float32).T  # [Cin, Cout]
        w2T = np.asarray(inputs[f"w2_{l}"]).astype(np.float32).T  # [Cin, Cout/4]
        b1 = np.asarray(inputs[f"b1_{l}"]).astype(np.float32)
        b2 = np.asarray(inputs[f"b2_{l}"]).astype(np.float32)
        C, Co = CS[l], COUT[l]
        for kk in range(KC[l]):
            rows = min(128, C - kk * 128)
            c0 = W1C[(l, kk)]
            wts[0:rows, c0:c0 + C] = w1T[kk * 128:kk * 128 + rows, :]
            c0 = W2C[(l, kk)]
            wts[0:rows, c0:c0 + Co] = w2T[kk * 128:kk * 128 + rows, :]
        for m in range(KC[l]):
            rows = min(128, C - m * 128)
            aux[0:rows, B1C[(l, m)]] = b1[m * 128:m * 128 + rows]
        aux[0:Co, B2C[l]] = b2
    aux[:, ICOL:ICOL + 128] = np.eye(128, dtype=np.float32)
    aux[:, OCOL:OCOL + 128] = 1.0
    aux[0, WVCOL:WVCOL + 16] = 1.0
    aux[0, WVCOL + 16:WVCOL + 20] = -1.0 / TAU

    # per-core x blobs: [NCH*128, 576] = packed [neigh | center] per K chunk
    xqs = [np.zeros((NCH * 128, 576), dtype=np_dt) for _ in range(NCORES)]
    xks = [np.zeros((NCH * 128, 576), dtype=np_dt) for _ in range(NCORES)]
    for l in range(4):
        C = CS[l]
        fq = np.asarray(inputs[f"fq{l}"])[:, :, :32, :32].reshape(NCORES, C, 1024)
        fk = np.asarray(inputs[f"fk{l}"])[:, :, :32, :32].reshape(NCORES, C, 1024)
        qn = fq[:, :, nidx[l]]   # [B, C, 512]
        qc = fq[:, :, cidx[l]]   # [B, C, 64]
        kn = fk[:, :, nidx[l]]
        kc_ = fk[:, :, cidx[l]]
        for b in range(NCORES):
            for kk in range(KC[l]):
                r0 = CHUNK[(l, kk)] * 128
                rows = min(128, C - kk * 128)
                sl = slice(kk * 128, kk * 128 + rows)
                xqs[b][r0:r0 + rows, 0:512] = qn[b, sl, :]
                xqs[b][r0:r0 + rows, 512:576] = qc[b, sl, :]
                xks[b][r0:r0 + rows, 0:512] = kn[b, sl, :]
                xks[b][r0:r0 + rows, 512:576] = kc_[b, sl, :]
    return wts, aux, xqs, xks


_LAST_RESULT = {}


def kernel(**inputs):
    assert int(inputs.get("start_layer", 0)) == 0
    assert int(inputs.get("end_layer", 4)) == 4
    assert int(inputs.get("num_s", 64)) == 64

    nc = _get_nc(F16)
    wts, aux, xqs, xks = _host_blobs(inputs, np.float16)
    auxh = np.ones((128, 128), dtype=np.float16)
    in_maps = [
        {"xq": xqs[b], "xk": xks[b], "wts": wts, "aux": aux, "auxh": auxh}
        for b in range(NCORES)
    ]
    r = run_bass_kernel_spmd(nc, in_maps, core_ids=list(range(NCORES)))
    _LAST_RESULT["r"] = r
    partials = [np.float64(r.results[b]["out"][0, 0]) for b in range(NCORES)]
    loss = np.float32(sum(partials) / (NCORES * S))
    return np.asarray(loss, dtype=np.float32)


# revision 27
# speedup vs baseline: 2.1165x; 1.3396x over previous
"""CCPL contrastive loss kernel for Trainium2 (8 NeuronCores, SPMD data-parallel over batch).

Contract: kernel(**inputs) takes the FULL unsharded inputs and returns the FULL
scalar loss (float32, shape ()).

Strategy
--------
Only the top-left 32x32 corner of each feature map is ever read (sid in [0,30),
neighborhood offsets in {0,1,2}).  The host performs *indexing only* (gather of
neighbor/center columns from the corner; no arithmetic) and uploads, per core:

  xq, xk : [8*128, 576] packed K-chunks of [neigh(512) | center(64)] columns
  wts    : [128, WTOT]  packed transposed MLP weights (w1T / w2T chunks)
  aux    : [128, 268]   b1/b2 columns, identity block, ones block

Core b processes batch b end-to-end on device:
  x = neigh - center                      (VectorE, stride-0 broadcast AP)
  h = relu(w1 @ x + b1)                   (TensorE + ScalarE)
  y = w2 @ h + b2                         (TensorE + VectorE)
  f = y / (||y||_2 + 1e-7)                (ones-matmul partition reduction,
                                           sqrt via exp(0.5*ln), VectorE recip)
  G = f_q^T f_k                           (TensorE, |G|<=1 so exp needs no max)
  CE = ln(sum_t exp(G/tau)) - G[s,s]/tau  (ScalarE exp+accum, diag via
                                           tensor_tensor_reduce with I128)
Per-core partial sum of CE returned as [1,1]; host sums 8 partials / (8*512).
"""

import numpy as np
from contextlib import ExitStack

import concourse.bass as bass
import concourse.bacc as bacc
import concourse.tile as tile
from concourse import mybir
from concourse.bass_utils import run_bass_kernel_spmd

F32 = mybir.dt.float32
F16 = mybir.dt.float16

# Force Exp/Ln/Relu to resolve to the one table set that contains all three
# (natural_log_exp_and_others), so the kernel pays a single ACT_TABLE_LOAD
# instead of thrashing between exp_and_others and natural_log (~1.3us each).
# Set ids stay aligned with act_info.json because only set CONTENTS are
# filtered, never the ordering.
_COMBINED_SET = "natural_log_exp_and_others"
_orig_get_tables = bacc.get_activation_tables


def _patched_get_tables(arch):
    t = _orig_get_tables(arch)
    strip = {
        mybir.ActivationFunctionType.Exp,
        mybir.ActivationFunctionType.Ln,
        mybir.ActivationFunctionType.Relu,
    }
    return {
        name: (fns if name == _COMBINED_SET else (set(fns) - strip))
        for name, fns in t.items()
    }


bacc.get_activation_tables = _patched_get_tables

TAU = 0.07
NCORES = 8
S = 512          # 8 * num_s samples per batch-layer
NS = 64          # num_s
CS = [64, 128, 256, 512]
COUT = [16, 32, 64, 128]
KC = [1, 1, 2, 4]                 # 128-row K chunks per layer
NCH = sum(KC)                     # 8 chunks total in the x blob
_DH = np.array([0, 0, 0, 1, 1, 2, 2, 2], dtype=np.int64)
_DW = np.array([0, 1, 2, 0, 2, 0, 1, 2], dtype=np.int64)

# chunk bookkeeping -----------------------------------------------------------
CHUNK = {}
_c = 0
for _l in range(4):
    for _kk in range(KC[_l]):
        CHUNK[(_l, _kk)] = _c
        _c += 1

# weight blob column offsets
W1C, W2C = {}, {}
_c = 0
for _l in range(4):
    for _kk in range(KC[_l]):
        W1C[(_l, _kk)] = _c
        _c += CS[_l]
for _l in range(4):
    for _kk in range(KC[_l]):
        W2C[(_l, _kk)] = _c
        _c += COUT[_l]
WTOT = _c

# aux blob layout (f32): cols 0..7 b1 chunks, 8..11 b2, 12..139 I128, 140..267 ones
B1C = {}
_c = 0
for _l in range(4):
    for _m in range(KC[_l]):
        B1C[(_l, _m)] = _c
        _c += 1
B2C = {l: 8 + l for l in range(4)}
ICOL = 12
OCOL = 140
WVCOL = 268          # row 0: [1.0]*16 | [-1/tau]*4  (final combine weights)
AUXW = 288


def _build_nc(dt_x=F16):
    nc = bacc.Bacc()
    xq = nc.dram_tensor("xq", [NCH * 128, 576], dt_x, kind="ExternalInput")
    xk = nc.dram_tensor("xk", [NCH * 128, 576], dt_x, kind="ExternalInput")
    wts = nc.dram_tensor("wts", [128, WTOT], dt_x, kind="ExternalInput")
    aux = nc.dram_tensor("aux", [128, AUXW], F32, kind="ExternalInput")
    auxh = nc.dram_tensor("auxh", [128, 128], F16, kind="ExternalInput")
    out = nc.dram_tensor("out", [1, 1], F32, kind="ExternalOutput")

    with ExitStack() as ctx:
        tc = ctx.enter_context(tile.TileContext(nc))
        const = ctx.enter_context(tc.tile_pool(name="const", bufs=1))
        work = ctx.enter_context(tc.tile_pool(name="work", bufs=2))
        ypool = ctx.enter_context(tc.tile_pool(name="ypool", bufs=4))
        fpool = ctx.enter_context(tc.tile_pool(name="fpool", bufs=4))
        # PSUM: mm1/mm2/small at 2 bufs x 1 bank = 6 banks, g at 1 buf x 2
        # banks = 2 banks -> exactly the 8 available banks
        ppool = ctx.enter_context(tc.tile_pool(name="psum", bufs=2, space="PSUM"))
        gpool = ctx.enter_context(tc.tile_pool(name="gpsum", bufs=1, space="PSUM"))

        wall = const.tile([128, WTOT], dt_x)
        nc.sync.dma_start(out=wall, in_=wts[:, :])
        aall = const.tile([128, AUXW], F32)
        nc.sync.dma_start(out=aall, in_=aux[:, :])
        xq_s = const.tile([128, NCH, 576], dt_x)
        nc.sync.dma_start(out=xq_s, in_=xq.rearrange("(n p) m -> p n m", p=128))
        xk_s = const.tile([128, NCH, 576], dt_x)
        nc.sync.dma_start(out=xk_s, in_=xk.rearrange("(n p) m -> p n m", p=128))

        hall = const.tile([128, 128], F16)
        nc.sync.dma_start(out=hall, in_=auxh[:, :])

        ones_col = aall[:, OCOL:OCOL + 1]
        # Z (row sums of exp(G/tau)) per G row-tile, one column per tile
        ZD = const.tile([128, 16], F32)
        # catb: cols 0..15 = per-tile sums of ln(Z); cols 16..19 = per-layer
        # sums of l_pos = sum(f_q * f_k)
        catb = const.tile([1, 20], F32)

        # x = neigh - center for ALL chunks of a branch in two fused DVE ops
        # (the [p, chunk, sample, neighbor] view has the center broadcast via
        # a stride-0 trailing AP dim)
        xsub = {}
        for bi, (xall, blob) in enumerate(((xq_s, xq), (xk_s, xk))):
            xs = const.tile([128, NCH, S], dt_x, tag=f"xsub{bi}")
            for half in range(2):
                csl = slice(half * 4, half * 4 + 4)
                in0 = xall[:, csl, 0:512].rearrange("p n (s j) -> p n s j", j=8)
                cb = xall[:, csl, 512:576]
                in1 = bass.AP(cb.tensor, cb.offset, [*cb.ap, [0, 8]])
                nc.vector.tensor_sub(
                    out=xs[:, csl, :].rearrange("p n (s j) -> p n s j", j=8),
                    in0=in0,
                    in1=in1,
                )
            xsub[bi] = xs

        for l in range(4):
            C, Co, K = CS[l], COUT[l], KC[l]
            ytiles = []
            for bi in range(2):
                xs = xsub[bi]
                # h = relu(w1 @ x + b1), per 128-row output chunk
                h = work.tile([128, K, S], dt_x, tag="h")
                for m in range(K):
                    rows = min(128, C - m * 128)
                    mm1 = ppool.tile([128, S], F32, tag="mm1")
                    for kk in range(K):
                        c0 = W1C[(l, kk)] + m * 128
                        nc.tensor.matmul(
                            mm1[0:rows, :],
                            lhsT=wall[:, c0:c0 + rows],
                            rhs=xs[:, CHUNK[(l, kk)], :],
                            start=(kk == 0),
                            stop=(kk == K - 1),
                        )
                    bc1 = B1C[(l, m)]
                    nc.scalar.activation(
                        out=h[0:rows, m, :],
                        in_=mm1[0:rows, :],
                        func=mybir.ActivationFunctionType.Relu,
                        bias=aall[0:rows, bc1:bc1 + 1],
                        scale=1.0,
                    )
                # y = w2 @ h + b2
                mm2 = ppool.tile([128, S], F32, tag="mm2")
                for kk in range(K):
                    rows = min(128, C - kk * 128)
                    c0 = W2C[(l, kk)]
                    nc.tensor.matmul(
                        mm2[0:Co, :],
                        lhsT=wall[0:rows, c0:c0 + Co],
                        rhs=h[0:rows, kk, :],
                        start=(kk == 0),
                        stop=(kk == K - 1),
                    )
                y = ypool.tile([128, S], F32, tag="y")
                nc.vector.tensor_scalar_add(
                    out=y[0:Co, :], in0=mm2[0:Co, :],
                    scalar1=aall[0:Co, B2C[l]:B2C[l] + 1],
                )
                ytiles.append(y)

            # squared col norms of both branches packed in the free dim of
            # one [1, 2, 512] PSUM tile (2 banks, both MMs partition-base 0)
            ssq = gpool.tile([1, 2, S], F32, tag="small")
            for bi in range(2):
                y2 = work.tile([128, S], F16, tag="y2")
                nc.gpsimd.tensor_mul(out=y2[0:Co, :], in0=ytiles[bi][0:Co, :],
                                     in1=ytiles[bi][0:Co, :])
                nc.tensor.matmul(
                    ssq[:, bi, :], lhsT=hall[0:Co, 0:1], rhs=y2[0:Co, :],
                    start=True, stop=True,
                )
            # rn = 1/sqrt(ssq) = exp(-0.5*ln(ssq)), both branches per ACT op
            t1 = work.tile([1, 2, S], F32, tag="t1")
            nc.scalar.activation(out=t1[:, :, :], in_=ssq[:, :, :],
                                 func=mybir.ActivationFunctionType.Ln)
            rn = work.tile([1, 2, S], F16, tag="rn")
            nc.scalar.activation(out=rn[:, :, :], in_=t1[:, :, :],
                                 func=mybir.ActivationFunctionType.Exp,
                                 scale=-0.5)
            # f = y * rn; rn row broadcast across partitions via K=1 ones
            # matmul (PSUM tile shares the "g" tag: lifetimes are disjoint)
            bc = gpool.tile([128, 2, S], F32, tag="g")
            ftiles = []
            for bi in range(2):
                nc.tensor.matmul(
                    bc[0:Co, bi, :], lhsT=hall[0:1, 0:Co], rhs=rn[:, bi, :],
                    start=True, stop=True,
                )
                f = fpool.tile([128, S], F16, tag="f")
                nc.vector.tensor_mul(out=f[0:Co, :], in0=ytiles[bi][0:Co, :],
                                     in1=bc[0:Co, bi, :])
                ftiles.append(f)

            fq_t, fk_t = ftiles
            # sum of positive logits: sum_s <f_q[:,s], f_k[:,s]>
            pprod = work.tile([128, S], F16, tag="pprod")
            nc.gpsimd.tensor_mul(out=pprod[0:Co, :], in0=fq_t[0:Co, :],
                                 in1=fk_t[0:Co, :])
            psum_pos = gpool.tile([1, 2, S], F32, tag="small")
            nc.tensor.matmul(psum_pos[:, 0, :], lhsT=hall[0:Co, 0:1],
                             rhs=pprod[0:Co, :], start=True, stop=True)
            nc.vector.reduce_sum(out=catb[:, 16 + l:17 + l],
                                 in_=psum_pos[:, 0, :],
                                 axis=mybir.AxisListType.X)
            # Gram tiles two at a time; one exp + one row-sum reduce per pair
            for half in range(2):
                g = gpool.tile([128, 2, S], F32, tag="g")
                for mm in range(2):
                    m = half * 2 + mm
                    nc.tensor.matmul(
                        g[:, mm, :],
                        lhsT=fq_t[0:Co, m * 128:(m + 1) * 128],
                        rhs=fk_t[0:Co, :],
                        start=True, stop=True,
                    )
                E = work.tile([128, 2, S], F32, tag="E")
                nc.scalar.activation(
                    out=E[:, :, :], in_=g[:, :, :],
                    func=mybir.ActivationFunctionType.Exp,
                    scale=1.0 / TAU,
                )
                i = l * 4 + half * 2
                nc.vector.reduce_sum(out=ZD[:, i:i + 2], in_=E[:, :, :],
                                     axis=mybir.AxisListType.X)

        # total = sum_{p,i} ln(Z) - (1/tau) * sum_l pos_l
        L = const.tile([128, 16], F32)
        nc.scalar.activation(out=L[:, :], in_=ZD[:, :],
                             func=mybir.ActivationFunctionType.Ln)
        tp = gpool.tile([1, 2, S], F32, tag="small")
        nc.tensor.matmul(tp[:, 0, 0:16], lhsT=ones_col, rhs=L[:, :],
                         start=True, stop=True)
        nc.vector.tensor_copy(out=catb[:, 0:16], in_=tp[:, 0, 0:16])
        wprod = const.tile([1, 20], F32)
        nc.vector.tensor_mul(out=wprod[:, :], in0=catb[:, :],
                             in1=aall[0:1, WVCOL:WVCOL + 20])
        res = const.tile([1, 1], F32)
        nc.vector.reduce_sum(out=res[:, :], in_=wprod[:, :], axis=mybir.AxisListType.X)
        nc.sync.dma_start(out=out[:, :], in_=res[:, :])
    # bass2jax's PJRT path serializes nc.m directly without finalizing;
    # Bacc's legalization passes (matmul wait splitting, register
    # allocation) only run inside finalize().
    nc.finalize()
    return nc


_NC_CACHE = {}


def _get_nc(dt_x=F16):
    key = str(dt_x)
    if key not in _NC_CACHE:
        _NC_CACHE[key] = _build_nc(dt_x)
    return _NC_CACHE[key]


def _host_blobs(inputs, np_dt=np.float16):
    """Build the shared wts/aux blobs and the per-core xq/xk blobs."""
    # gather indices per layer (host-side indexing only)
    nidx, cidx = [], []
    for l in range(4):
        sid = np.asarray(inputs[f"sid{l}"]).astype(np.int64)
        nidx.append(((sid[:, 0:1] + _DH) * 32 + (sid[:, 1:2] + _DW)).reshape(-1))
        cidx.append((sid[:, 0] + 1) * 32 + (sid[:, 1] + 1))

    wts = np.zeros((128, WTOT), dtype=np_dt)
    aux = np.zeros((128, AUXW), dtype=np.float32)
    for l in range(4):
        w1T = np.asarray(inputs[f"w1_{l}"]).astype(np.float32).T  # [Cin, Cout]
        w2T = np.asarray(inputs[f"w2_{l}"]).astype(np.float32).T  # [Cin, Cout/4]
        b1 = np.asarray(inputs[f"b1_{l}"]).astype(np.float32)
        b2 = np.asarray(inputs[f"b2_{l}"]).astype(np.float32)
        C, Co = CS[l], COUT[l]
        for kk in range(KC[l]):
            rows = min(128, C - kk * 128)
            c0 = W1C[(l, kk)]
            wts[0:rows, c0:c0 + C] = w1T[kk * 128:kk * 128 + rows, :]
            c0 = W2C[(l, kk)]
            wts[0:rows, c0:c0 + Co] = w2T[kk * 128:kk * 128 + rows, :]
        for m in range(KC[l]):
            rows = min(128, C - m * 128)
            aux[0:rows, B1C[(l, m)]] = b1[m * 128:m * 128 + rows]
        aux[0:Co, B2C[l]] = b2
    aux[:, ICOL:ICOL + 128] = np.eye(128, dtype=np.float32)
    aux[:, OCOL:OCOL + 128] = 1.0
    aux[0, WVCOL:WVCOL + 16] = 1.0
    aux[0, WVCOL + 16:WVCOL + 20] = -1.0 / TAU

    # per-core x blobs: [NCH*128, 576] = packed [neigh | center] per K chunk
    xqs = [np.zeros((NCH * 128, 576), dtype=np_dt) for _ in range(NCORES)]
    xks = [np.zeros((NCH * 128, 576), dtype=np_dt) for _ in range(NCORES)]
    for l in range(4):
        C = CS[l]
        fq = np.asarray(inputs[f"fq{l}"])[:, :, :32, :32].reshape(NCORES, C, 1024)
        fk = np.asarray(inputs[f"fk{l}"])[:, :, :32, :32].reshape(NCORES, C, 1024)
        qn = fq[:, :, nidx[l]]   # [B, C, 512]
        qc = fq[:, :, cidx[l]]   # [B, C, 64]
        kn = fk[:, :, nidx[l]]
        kc_ = fk[:, :, cidx[l]]
        for b in range(NCORES):
            for kk in range(KC[l]):
                r0 = CHUNK[(l, kk)] * 128
                rows = min(128, C - kk * 128)
                sl = slice(kk * 128, kk * 128 + rows)
                xqs[b][r0:r0 + rows, 0:512] = qn[b, sl, :]
                xqs[b][r0:r0 + rows, 512:576] = qc[b, sl, :]
                xks[b][r0:r0 + rows, 0:512] = kn[b, sl, :]
                xks[b][r0:r0 + rows, 512:576] = kc_[b, sl, :]
    return wts, aux, xqs, xks


_LAST_RESULT = {}


def kernel(**inputs):
    assert int(inputs.get("start_layer", 0)) == 0
    assert int(inputs.get("end_layer", 4)) == 4
    assert int(inputs.get("num_s", 64)) == 64

    nc = _get_nc(F16)
    wts, aux, xqs, xks = _host_blobs(inputs, np.float16)
    auxh = np.ones((128, 128), dtype=np.float16)
    in_maps = [
        {"xq": xqs[b], "xk": xks[b], "wts": wts, "aux": aux, "auxh": auxh}
        for b in range(NCORES)
    ]
    r = run_bass_kernel_spmd(nc, in_maps, core_ids=list(range(NCORES)))
    _LAST_RESULT["r"] = r
    partials = [np.float64(r.results[b]["out"][0, 0]) for b in range(NCORES)]
    loss = np.float32(sum(partials) / (NCORES * S))
    return np.asarray(loss, dtype=np.float32)


# revision 32
# speedup vs baseline: 2.2116x; 1.0450x over previous
"""CCPL contrastive loss kernel for Trainium2 (8 NeuronCores, SPMD data-parallel over batch).

Contract: kernel(**inputs) takes the FULL unsharded inputs and returns the FULL
scalar loss (float32, shape ()).

Strategy
--------
Only the top-left 32x32 corner of each feature map is ever read (sid in [0,30),
neighborhood offsets in {0,1,2}).  The host performs *indexing only* (gather of
neighbor/center columns from the corner; no arithmetic) and uploads, per core:

  xq, xk : [8*128, 576] packed K-chunks of [neigh(512) | center(64)] columns
  wts    : [128, WTOT]  packed transposed MLP weights (w1T / w2T chunks)
  aux    : [128, 268]   b1/b2 columns, identity block, ones block

Core b processes batch b end-to-end on device:
  x = neigh - center                      (VectorE, stride-0 broadcast AP)
  h = relu(w1 @ x + b1)                   (TensorE + ScalarE)
  y = w2 @ h + b2                         (TensorE + VectorE)
  f = y / (||y||_2 + 1e-7)                (ones-matmul partition reduction,
                                           sqrt via exp(0.5*ln), VectorE recip)
  G = f_q^T f_k                           (TensorE, |G|<=1 so exp needs no max)
  CE = ln(sum_t exp(G/tau)) - G[s,s]/tau  (ScalarE exp+accum, diag via
                                           tensor_tensor_reduce with I128)
Per-core partial sum of CE returned as [1,1]; host sums 8 partials / (8*512).
"""

import numpy as np
from contextlib import ExitStack

import concourse.bass as bass
import concourse.bacc as bacc
import concourse.tile as tile
from concourse import mybir
from concourse.bass_utils import run_bass_kernel_spmd

F32 = mybir.dt.float32
F16 = mybir.dt.float16

# Force Exp/Ln/Relu to resolve to the one table set that contains all three
# (natural_log_exp_and_others), so the kernel pays a single ACT_TABLE_LOAD
# instead of thrashing between exp_and_others and natural_log (~1.3us each).
# Set ids stay aligned with act_info.json because only set CONTENTS are
# filtered, never the ordering.
_COMBINED_SET = "natural_log_exp_and_others"
_orig_get_tables = bacc.get_activation_tables


def _patched_get_tables(arch):
    t = _orig_get_tables(arch)
    strip = {
        mybir.ActivationFunctionType.Exp,
        mybir.ActivationFunctionType.Ln,
        mybir.ActivationFunctionType.Relu,
    }
    return {
        name: (fns if name == _COMBINED_SET else (set(fns) - strip))
        for name, fns in t.items()
    }


bacc.get_activation_tables = _patched_get_tables

TAU = 0.07
NCORES = 8
S = 512          # 8 * num_s samples per batch-layer
NS = 64          # num_s
CS = [64, 128, 256, 512]
COUT = [16, 32, 64, 128]
KC = [1, 1, 2, 4]                 # 128-row K chunks per layer
NCH = sum(KC)                     # 8 chunks total in the x blob
_DH = np.array([0, 0, 0, 1, 1, 2, 2, 2], dtype=np.int64)
_DW = np.array([0, 1, 2, 0, 2, 0, 1, 2], dtype=np.int64)

# chunk bookkeeping -----------------------------------------------------------
CHUNK = {}
_c = 0
for _l in range(4):
    for _kk in range(KC[_l]):
        CHUNK[(_l, _kk)] = _c
        _c += 1

# weight blob column offsets
W1C, W2C = {}, {}
_c = 0
for _l in range(4):
    for _kk in range(KC[_l]):
        W1C[(_l, _kk)] = _c
        _c += CS[_l]
for _l in range(4):
    for _kk in range(KC[_l]):
        W2C[(_l, _kk)] = _c
        _c += COUT[_l]
WTOT = _c

# aux blob layout (f32): cols 0..7 b1 chunks, 8..11 b2, 12..139 I128, 140..267 ones
B1C = {}
_c = 0
for _l in range(4):
    for _m in range(KC[_l]):
        B1C[(_l, _m)] = _c
        _c += 1
B2C = {l: 8 + l for l in range(4)}
ICOL = 12
OCOL = 140
WVCOL = 268          # row 0: [1.0]*16 | [-1/tau]*4  (final combine weights)
AUXW = 288


def _build_nc(dt_x=F16):
    nc = bacc.Bacc()
    xq = nc.dram_tensor("xq", [NCH * 128, 576], dt_x, kind="ExternalInput")
    xk = nc.dram_tensor("xk", [NCH * 128, 576], dt_x, kind="ExternalInput")
    wts = nc.dram_tensor("wts", [128, WTOT], dt_x, kind="ExternalInput")
    aux = nc.dram_tensor("aux", [128, AUXW], F32, kind="ExternalInput")
    auxh = nc.dram_tensor("auxh", [128, 128], F16, kind="ExternalInput")
    out = nc.dram_tensor("out", [1, 1], F32, kind="ExternalOutput")

    with ExitStack() as ctx:
        tc = ctx.enter_context(tile.TileContext(nc))
        const = ctx.enter_context(tc.tile_pool(name="const", bufs=1))
        work = ctx.enter_context(tc.tile_pool(name="work", bufs=2))
        hpool = ctx.enter_context(tc.tile_pool(name="hpool", bufs=3))
        ypool = ctx.enter_context(tc.tile_pool(name="ypool", bufs=6))
        fpool = ctx.enter_context(tc.tile_pool(name="fpool", bufs=6))
        # PSUM: mm1/mm2/small at 2 bufs x 1 bank = 6 banks, g at 1 buf x 2
        # banks = 2 banks -> exactly the 8 available banks
        ppool = ctx.enter_context(tc.tile_pool(name="psum", bufs=2, space="PSUM"))
        gpool = ctx.enter_context(tc.tile_pool(name="gpsum", bufs=1, space="PSUM"))

        wall = const.tile([128, WTOT], dt_x)
        nc.sync.dma_start(out=wall, in_=wts[:, :])
        aall = const.tile([128, AUXW], F32)
        nc.sync.dma_start(out=aall, in_=aux[:, :])
        # land layer-3 chunks (rows 4*128..) first: layers are processed
        # big-first, so the PE gets dense work as early as possible
        xq_s = const.tile([128, NCH, 576], dt_x)
        xk_s = const.tile([128, NCH, 576], dt_x)
        for s_t, blob in ((xq_s, xq), (xk_s, xk)):
            r = blob.rearrange("(n p) m -> p n m", p=128)
            nc.sync.dma_start(out=s_t[:, 4:8, :], in_=r[:, 4:8, :])
            nc.sync.dma_start(out=s_t[:, 0:4, :], in_=r[:, 0:4, :])

        hall = const.tile([128, 128], F16)
        nc.sync.dma_start(out=hall, in_=auxh[:, :])

        ones_col = aall[:, OCOL:OCOL + 1]
        # Z (row sums of exp(G/tau)) per G row-tile, one column per tile
        ZD = const.tile([128, 16], F32)
        # catb: cols 0..15 = per-tile sums of ln(Z); cols 16..19 = per-layer
        # sums of l_pos = sum(f_q * f_k)
        catb = const.tile([1, 20], F32)

        # x = neigh - center for ALL chunks of a branch in two fused DVE ops
        # (the [p, chunk, sample, neighbor] view has the center broadcast via
        # a stride-0 trailing AP dim)
        xsub = {}
        for bi, (xall, blob) in enumerate(((xq_s, xq), (xk_s, xk))):
            xs = const.tile([128, NCH, S], dt_x, tag=f"xsub{bi}")
            for half in (1, 0):
                csl = slice(half * 4, half * 4 + 4)
                in0 = xall[:, csl, 0:512].rearrange("p n (s j) -> p n s j", j=8)
                cb = xall[:, csl, 512:576]
                in1 = bass.AP(cb.tensor, cb.offset, [*cb.ap, [0, 8]])
                nc.vector.tensor_sub(
                    out=xs[:, csl, :].rearrange("p n (s j) -> p n s j", j=8),
                    in0=in0,
                    in1=in1,
                )
            xsub[bi] = xs

        for l in (3, 2, 1, 0):
            C, Co, K = CS[l], COUT[l], KC[l]
            ytiles = []
            for bi in range(2):
                xs = xsub[bi]
                # h = relu(w1 @ x + b1), per 128-row output chunk
                h = hpool.tile([128, K, S], dt_x, tag="h")
                for m in range(K):
                    rows = min(128, C - m * 128)
                    mm1 = ppool.tile([128, S], F32, tag="mm1")
                    for kk in range(K):
                        c0 = W1C[(l, kk)] + m * 128
                        nc.tensor.matmul(
                            mm1[0:rows, :],
                            lhsT=wall[:, c0:c0 + rows],
                            rhs=xs[:, CHUNK[(l, kk)], :],
                            start=(kk == 0),
                            stop=(kk == K - 1),
                        )
                    bc1 = B1C[(l, m)]
                    nc.scalar.activation(
                        out=h[0:rows, m, :],
                        in_=mm1[0:rows, :],
                        func=mybir.ActivationFunctionType.Relu,
                        bias=aall[0:rows, bc1:bc1 + 1],
                        scale=1.0,
                    )
                # y = w2 @ h + b2
                mm2 = ppool.tile([128, S], F32, tag="mm2")
                for kk in range(K):
                    rows = min(128, C - kk * 128)
                    c0 = W2C[(l, kk)]
                    nc.tensor.matmul(
                        mm2[0:Co, :],
                        lhsT=wall[0:rows, c0:c0 + Co],
                        rhs=h[0:rows, kk, :],
                        start=(kk == 0),
                        stop=(kk == K - 1),
                    )
                y = ypool.tile([128, S], F32, tag="y")
                nc.vector.tensor_scalar_add(
                    out=y[0:Co, :], in0=mm2[0:Co, :],
                    scalar1=aall[0:Co, B2C[l]:B2C[l] + 1],
                )
                ytiles.append(y)

            # squared col norms of both branches packed in the free dim of
            # one [1, 2, 512] PSUM tile (2 banks, both MMs partition-base 0)
            ssq = gpool.tile([1, 2, S], F32, tag="small")
            for bi in range(2):
                y2 = work.tile([128, S], F16, tag="y2")
                nc.gpsimd.tensor_mul(out=y2[0:Co, :], in0=ytiles[bi][0:Co, :],
                                     in1=ytiles[bi][0:Co, :])
                nc.tensor.matmul(
                    ssq[:, bi, :], lhsT=hall[0:Co, 0:1], rhs=y2[0:Co, :],
                    start=True, stop=True,
                )
            # rn = 1/sqrt(ssq) = exp(-0.5*ln(ssq)), both branches per ACT op
            t1 = work.tile([1, 2, S], F32, tag="t1")
            nc.scalar.activation(out=t1[:, :, :], in_=ssq[:, :, :],
                                 func=mybir.ActivationFunctionType.Ln)
            rn = work.tile([1, 2, S], F16, tag="rn")
            nc.scalar.activation(out=rn[:, :, :], in_=t1[:, :, :],
                                 func=mybir.ActivationFunctionType.Exp,
                                 scale=-0.5)
            # f = y * rn; rn row broadcast across partitions via K=1 ones
            # matmul (PSUM tile shares the "g" tag: lifetimes are disjoint)
            bc = gpool.tile([128, 2, S], F32, tag="g")
            ftiles = []
            for bi in range(2):
                nc.tensor.matmul(
                    bc[0:Co, bi, :], lhsT=hall[0:1, 0:Co], rhs=rn[:, bi, :],
                    start=True, stop=True,
                )
                f = fpool.tile([128, S], F16, tag="f")
                nc.vector.tensor_mul(out=f[0:Co, :], in0=ytiles[bi][0:Co, :],
                                     in1=bc[0:Co, bi, :])
                ftiles.append(f)

            fq_t, fk_t = ftiles
            # sum of positive logits: sum_s <f_q[:,s], f_k[:,s]>
            pprod = work.tile([128, S], F16, tag="pprod")
            nc.gpsimd.tensor_mul(out=pprod[0:Co, :], in0=fq_t[0:Co, :],
                                 in1=fk_t[0:Co, :])
            psum_pos = gpool.tile([1, 2, S], F32, tag="small")
            nc.tensor.matmul(psum_pos[:, 0, :], lhsT=hall[0:Co, 0:1],
                             rhs=pprod[0:Co, :], start=True, stop=True)
            nc.vector.reduce_sum(out=catb[:, 16 + l:17 + l],
                                 in_=psum_pos[:, 0, :],
                                 axis=mybir.AxisListType.X)
            # Gram tiles two at a time; one exp + one row-sum reduce per pair
            for half in range(2):
                g = gpool.tile([128, 2, S], F32, tag="g")
                for mm in range(2):
                    m = half * 2 + mm
                    nc.tensor.matmul(
                        g[:, mm, :],
                        lhsT=fq_t[0:Co, m * 128:(m + 1) * 128],
                        rhs=fk_t[0:Co, :],
                        start=True, stop=True,
                    )
                E = work.tile([128, 2, S], F32, tag="E")
                nc.scalar.activation(
                    out=E[:, :, :], in_=g[:, :, :],
                    func=mybir.ActivationFunctionType.Exp,
                    scale=1.0 / TAU,
                )
                i = l * 4 + half * 2
                nc.vector.reduce_sum(out=ZD[:, i:i + 2], in_=E[:, :, :],
                                     axis=mybir.AxisListType.X)

        # total = sum_{p,i} ln(Z) - (1/tau) * sum_l pos_l
        L = const.tile([128, 16], F32)
        nc.scalar.activation(out=L[:, :], in_=ZD[:, :],
                             func=mybir.ActivationFunctionType.Ln)
        tp = gpool.tile([1, 2, S], F32, tag="small")
        nc.tensor.matmul(tp[:, 0, 0:16], lhsT=ones_col, rhs=L[:, :],
                         start=True, stop=True)
        nc.vector.tensor_copy(out=catb[:, 0:16], in_=tp[:, 0, 0:16])
        wprod = const.tile([1, 20], F32)
        nc.vector.tensor_mul(out=wprod[:, :], in0=catb[:, :],
                             in1=aall[0:1, WVCOL:WVCOL + 20])
        res = const.tile([1, 1], F32)
        nc.vector.reduce_sum(out=res[:, :], in_=wprod[:, :], axis=mybir.AxisListType.X)
        nc.sync.dma_start(out=out[:, :], in_=res[:, :])
    # bass2jax's PJRT path serializes nc.m directly without finalizing;
    # Bacc's legalization passes (matmul wait splitting, register
    # allocation) only run inside finalize().
    nc.finalize()
    return nc


_NC_CACHE = {}


def _get_nc(dt_x=F16):
    key = str(dt_x)
    if key not in _NC_CACHE:
        _NC_CACHE[key] = _build_nc(dt_x)
    return _NC_CACHE[key]


def _host_blobs(inputs, np_dt=np.float16):
    """Build the shared wts/aux blobs and the per-core xq/xk blobs."""
    # gather indices per layer (host-side indexing only)
    nidx, cidx = [], []
    for l in range(4):
        sid = np.asarray(inputs[f"sid{l}"]).astype(np.int64)
        nidx.append(((sid[:, 0:1] + _DH) * 32 + (sid[:, 1:2] + _DW)).reshape(-1))
        cidx.append((sid[:, 0] + 1) * 32 + (sid[:, 1] + 1))

    wts = np.zeros((128, WTOT), dtype=np_dt)
    aux = np.zeros((128, AUXW), dtype=np.float32)
    for l in range(4):
        w1T = np.asarray(inputs[f"w1_{l}"]).astype(np.float32).T  # [Cin, Cout]
        w2T = np.asarray(inputs[f"w2_{l}"]).astype(np.float32).T  # [Cin, Cout/4]
        b1 = np.asarray(inputs[f"b1_{l}"]).astype(np.float32)
        b2 = np.asarray(inputs[f"b2_{l}"]).astype(np.float32)
        C, Co = CS[l], COUT[l]
        for kk in range(KC[l]):
            rows = min(128, C - kk * 128)
            c0 = W1C[(l, kk)]
            wts[0:rows, c0:c0 + C] = w1T[kk * 128:kk * 128 + rows, :]
            c0 = W2C[(l, kk)]
            wts[0:rows, c0:c0 + Co] = w2T[kk * 128:kk * 128 + rows, :]
        for m in range(KC[l]):
            rows = min(128, C - m * 128)
            aux[0:rows, B1C[(l, m)]] = b1[m * 128:m * 128 + rows]
        aux[0:Co, B2C[l]] = b2
    aux[:, ICOL:ICOL + 128] = np.eye(128, dtype=np.float32)
    aux[:, OCOL:OCOL + 128] = 1.0
    aux[0, WVCOL:WVCOL + 16] = 1.0
    aux[0, WVCOL + 16:WVCOL + 20] = -1.0 / TAU

    # per-core x blobs: [NCH*128, 576] = packed [neigh | center] per K chunk
    xqs = [np.zeros((NCH * 128, 576), dtype=np_dt) for _ in range(NCORES)]
    xks = [np.zeros((NCH * 128, 576), dtype=np_dt) for _ in range(NCORES)]
    for l in range(4):
        C = CS[l]
        fq = np.asarray(inputs[f"fq{l}"])[:, :, :32, :32].reshape(NCORES, C, 1024)
        fk = np.asarray(inputs[f"fk{l}"])[:, :, :32, :32].reshape(NCORES, C, 1024)
        qn = fq[:, :, nidx[l]]   # [B, C, 512]
        qc = fq[:, :, cidx[l]]   # [B, C, 64]
        kn = fk[:, :, nidx[l]]
        kc_ = fk[:, :, cidx[l]]
        for b in range(NCORES):
            for kk in range(KC[l]):
                r0 = CHUNK[(l, kk)] * 128
                rows = min(128, C - kk * 128)
                sl = slice(kk * 128, kk * 128 + rows)
                xqs[b][r0:r0 + rows, 0:512] = qn[b, sl, :]
                xqs[b][r0:r0 + rows, 512:576] = qc[b, sl, :]
                xks[b][r0:r0 + rows, 0:512] = kn[b, sl, :]
                xks[b][r0:r0 + rows, 512:576] = kc_[b, sl, :]
    return wts, aux, xqs, xks


_LAST_RESULT = {}


def kernel(**inputs):
    assert int(inputs.get("start_layer", 0)) == 0
    assert int(inputs.get("end_layer", 4)) == 4
    assert int(inputs.get("num_s", 64)) == 64

    nc = _get_nc(F16)
    wts, aux, xqs, xks = _host_blobs(inputs, np.float16)
    auxh = np.ones((128, 128), dtype=np.float16)
    in_maps = [
        {"xq": xqs[b], "xk": xks[b], "wts": wts, "aux": aux, "auxh": auxh}
        for b in range(NCORES)
    ]
    r = run_bass_kernel_spmd(nc, in_maps, core_ids=list(range(NCORES)))
    _LAST_RESULT["r"] = r
    partials = [np.float64(r.results[b]["out"][0, 0]) for b in range(NCORES)]
    loss = np.float32(sum(partials) / (NCORES * S))
    return np.asarray(loss, dtype=np.float32)
